# revision 19
# baseline (speedup 1.0000x reference)
"""CoedgeConvLayer Trainium2 kernel.

y = relu(x @ W_self + x[next] @ W_next + x[prev] @ W_prev + x[mate] @ W_mate + b_tot)

Sharding: rows (coedges) data-parallel across 8 NeuronCores; the full feature
table is replicated per core so neighbor gathers are purely local (no
collectives).  The SPMD program is identical on every core — all per-core
differences live in the index data.  Row mapping is natural: local row
r = b*G*128 + g*128 + p lives on partition p of subtile (b, g).

v2 design (vs the f32 per-subtile-gather baseline):
- bf16 features/weights (fp32 PSUM accumulate): 4x faster PE matmuls, half
  the gather traffic.  rel err ~2.5e-3, well under the 2e-2 gate.
- One batched indirect gather per block (3 neighbor streams x G subtiles =
  21 row-sets, 2688 descriptors) instead of 28 separate ones: SWDGE fixed
  overhead (~1us/instr serialized on the Pool engine) drops 28x.  This was
  the baseline's actual bottleneck.
- Self stream: rows are consecutive, so it skips the gather entirely and is
  loaded pre-transposed by the DMA XBAR (dma transpose DRAM->SBUF, bf16),
  eliminating its PE transposes and DVE copies.
- Neighbor subtiles are transposed on PE (identity matmul) into a per-stream
  PSUM tile [128, 256] and moved to SBUF with one DVE copy per stream.
- 8 accumulating matmuls (3 nbr streams x 2 K-chunks + self x 2) plus a K=1
  bias outer product run into PSUM; fused ReLU copy (ACT) to SBUF, then one
  block store.
"""

import os

import numpy as np

import concourse.bass as bass
from concourse import bacc
import concourse.mybir as mybir
import concourse.tile as tile
from concourse import bass_utils
from concourse.masks import make_identity

# Problem constants (hardcoded per harness contract).
N = 200000
D = 256
NCORES = 8
ROWS_PER_CORE = N // NCORES          # 25000
P = 128
SUBTILES = (ROWS_PER_CORE + P - 1) // P   # 196
PAD_ROWS = SUBTILES * P              # 25088
G = 7                                # subtiles per block
NBLOCKS = SUBTILES // G              # 28
KCHUNKS = 2                          # 256 = 2 * 128
NNBR = 3                             # next, prev, mate
# Feature rows padded so the self-stream block reads never go out of
# bounds (last core's padded rows reach N%NCORES short of base+PAD_ROWS).
NPAD = ((NCORES - 1) * ROWS_PER_CORE + PAD_ROWS + P - 1) // P * P  # 200192

USE_BF16 = os.environ.get("KERNEL_BF16", "1") == "1"
ABLATE = os.environ.get("KERNEL_ABLATE", "")
GBUFS = int(os.environ.get("KERNEL_GBUFS", "3"))
DESIGN = os.environ.get("KERNEL_DESIGN", "v2")
# Debug: limit the number of blocks actually computed (output beyond is junk).
DBG_NBLOCKS = int(os.environ.get("KERNEL_NBLOCKS", "0")) or None

# g2 region bucketing: dma_gather uses int16 indices, so neighbor rows are
# bucketed by 32768-row table region (region-relative index fits int16) and
# gathered pre-transposed; phase B un-sorts via a second int16 dma_gather
# from a DRAM staging buffer holding the transformed rows.
REGW = 32768
NREG = (NPAD + REGW - 1) // REGW                     # 7
# Per-region capacity (multiple of 128, >= binomial count + 8 sigma).
CAPS = [4480] * (NREG - 1) + [768]
RBASE = np.concatenate([[0], np.cumsum(CAPS)]).astype(np.int64)
TOTPOS = int(RBASE[-1])                              # 27648 < 32768
assert TOTPOS < 32768

if USE_BF16:
    import ml_dtypes
    _FEAT_DT = mybir.dt.bfloat16
    _FEAT_NP = ml_dtypes.bfloat16
else:
    _FEAT_DT = mybir.dt.float32
    _FEAT_NP = np.float32


def _build_nc(repeat=1):
    if DESIGN == "g2":
        return _build_nc_g2(repeat)
    return _build_nc_v2(repeat)


def _build_nc_v2(repeat=1):
    nc = bacc.Bacc("TRN2", debug=False, enable_partition_id=False)
    f32 = mybir.dt.float32
    feats = nc.dram_tensor("features", [NPAD, D], _FEAT_DT,
                           kind="ExternalInput")
    w = nc.dram_tensor("w", [4 * D, D], _FEAT_DT, kind="ExternalInput")
    bias = nc.dram_tensor("bias", [1, D], _FEAT_DT, kind="ExternalInput")
    idx = nc.dram_tensor("idx", [P, NBLOCKS * NNBR * G], mybir.dt.int32,
                         kind="ExternalInput")
    out = nc.dram_tensor("out", [PAD_ROWS, D], f32, kind="ExternalOutput")

    feats_ap = feats.ap()
    out_ap = out.ap()
    SG = NNBR * G                    # neighbor index columns per block

    with tile.TileContext(nc) as tc:
        with (
            tc.tile_pool(name="const", bufs=1) as cpool,
            tc.tile_pool(name="selfp", bufs=2) as spool,
            tc.tile_pool(name="gather", bufs=GBUFS) as gpool,
            tc.tile_pool(name="xt", bufs=6) as xtpool,
            tc.tile_pool(name="outp", bufs=2) as opool,
            tc.tile_pool(name="pt", bufs=6, space="PSUM") as ptpool,
            tc.tile_pool(name="pacc", bufs=2, space="PSUM") as paccpool,
        ):
            # Resident constants.
            w_sb = cpool.tile([P, 4 * KCHUNKS, D], _FEAT_DT)
            nc.sync.dma_start(
                out=w_sb[:], in_=w.ap().rearrange("(c p) n -> p c n", p=P))
            bias_sb = cpool.tile([1, D], _FEAT_DT)
            nc.sync.dma_start(out=bias_sb[:], in_=bias.ap())
            idx_sb = cpool.tile([P, NBLOCKS * SG], mybir.dt.int32)
            nc.sync.dma_start(out=idx_sb[:], in_=idx.ap())
            ident = cpool.tile([P, P], _FEAT_DT)
            make_identity(nc, ident[:])
            ones_sb = cpool.tile([1, P], _FEAT_DT)
            nc.gpsimd.memset(ones_sb[:], 1.0)
            # Priming transpose: folds the gpsimd-preamble wait into PE's
            # vector clock so steady-state PE instructions need at most one
            # sem wait (the lowered LDWEIGHTS struct has a single wait slot).
            pt0 = ptpool.tile([P, P], _FEAT_DT, tag='pt')
            nc.tensor.transpose(pt0[:], ident[:], ident[:])

            for b in range((DBG_NBLOCKS or NBLOCKS) * repeat):
                b = b % NBLOCKS
                r0 = b * G * P
                # Neighbor gathers.  The HW SWDGE ucode for dynamic DMA
                # strictly supports ONE index per partition per instruction
                # (verified empirically: multi-column offset APs stream
                # dest_size/128 contiguous elements from idx[p, 0] instead),
                # so this is 3 instructions per subtile, 128 rows each.
                xg = gpool.tile([P, SG, D], _FEAT_DT, tag="xg")
                for s in range(NNBR):
                    for g in range(G):
                        col = b * SG + s * G + g
                        nc.gpsimd.indirect_dma_start(
                            out=xg[:, s * G + g, :],
                            out_offset=None,
                            in_=feats_ap,
                            in_offset=bass.IndirectOffsetOnAxis(
                                ap=idx_sb[:, col:col + 1], axis=0),
                        )
                # Self stream, pre-transposed by the DMA XBAR straight from
                # DRAM: [896 rows, 128 f] -> [128 f, 896 rows] per K-chunk.
                xself = []
                for ki in range(KCHUNKS):
                    xs = spool.tile([P, G * P], _FEAT_DT, tag="xself")
                    nc.sync.dma_start(
                        out=xs[:],
                        in_=feats_ap[r0:r0 + G * P, ki * P:(ki + 1) * P],
                        transpose=True)
                    xself.append(xs)
                outsb = opool.tile([P, G, D], mybir.dt.float32)
                for g in range(G):
                    # Transpose the 3 neighbor subtiles so d_in lands on
                    # partitions; one PSUM tile + one DVE copy per stream.
                    xts = []
                    for s in range(NNBR):
                        pt = ptpool.tile([P, KCHUNKS * P], _FEAT_DT, tag='pt')
                        src = xg[:, s * G + g, :]
                        for ki in range(KCHUNKS):
                            nc.tensor.transpose(
                                pt[:, ki * P:(ki + 1) * P],
                                src[:, ki * P:(ki + 1) * P], ident[:])
                        xt = xtpool.tile([P, KCHUNKS * P], _FEAT_DT)
                        nc.vector.tensor_copy(out=xt[:], in_=pt[:])
                        xts.append(xt)
                    pacc = paccpool.tile([P, D], mybir.dt.float32)
                    # Self stream: 2 K-chunk matmuls from the XBAR tiles.
                    first = True
                    for ki in range(KCHUNKS):
                        if ABLATE and "self" not in ABLATE:
                            break
                        nc.tensor.matmul(
                            pacc[:], lhsT=xself[ki][:, g * P:(g + 1) * P],
                            rhs=w_sb[:, ki, :],
                            start=first, stop=False)
                        first = False
                    # Neighbor streams: 6 accumulating matmuls.
                    for s in range(NNBR):
                        if ABLATE and f"n{s}" not in ABLATE:
                            continue
                        for ki in range(KCHUNKS):
                            nc.tensor.matmul(
                                pacc[:], lhsT=xts[s][:, ki * P:(ki + 1) * P],
                                rhs=w_sb[:, (s + 1) * KCHUNKS + ki, :],
                                start=first, stop=False)
                            first = False
                    # Bias as a K=1 outer product: ones[128] x b_tot[256].
                    nc.tensor.matmul(
                        pacc[:], lhsT=ones_sb[:1, :], rhs=bias_sb[:1, :],
                        start=False, stop=True)
                    # Fused ReLU on the PSUM -> SBUF move.
                    nc.scalar.activation(
                        outsb[:, g, :], pacc[:],
                        mybir.ActivationFunctionType.Relu)
                nc.sync.dma_start(
                    out=out_ap[r0:r0 + G * P, :].rearrange(
                        "(g p) n -> p g n", p=P),
                    in_=outsb[:],
                )
    nc.compile()
    return nc


def _build_nc_g2(repeat=1):
    """Gather-transform-stage-regather design: no PE transposes, no
    per-subtile SWDGE tax.  Phase A per neighbor stream: region-bucketed
    transposed dma_gather (features land d_in-on-partitions), 2 accumulating
    matmuls per 128 rows, PSUM->SBUF copy (bf16), sequential store to DRAM
    staging in bucket order.  Phase B per block: 3 un-sorting dma_gathers
    (non-transpose, position indices), self stream via DMA-XBAR transpose
    straight from DRAM, bias outer product, two batched DVE adds + one
    PSUM add, fused ReLU to SBUF, block store."""
    nc = bacc.Bacc("TRN2", debug=False, enable_partition_id=False)
    f32 = mybir.dt.float32
    i16 = mybir.dt.int16
    feats = nc.dram_tensor("features", [NPAD, D], _FEAT_DT,
                           kind="ExternalInput")
    w = nc.dram_tensor("w", [4 * D, D], _FEAT_DT, kind="ExternalInput")
    bias = nc.dram_tensor("bias", [1, D], _FEAT_DT, kind="ExternalInput")
    NI16 = TOTPOS // 16
    NP16 = PAD_ROWS // 16
    idx16 = nc.dram_tensor("idx16", [P, NNBR * NI16], i16,
                           kind="ExternalInput")
    pos16 = nc.dram_tensor("pos16", [P, NNBR * NP16], i16,
                           kind="ExternalInput")
    out = nc.dram_tensor("out", [PAD_ROWS, D], f32, kind="ExternalOutput")

    feats_ap = feats.ap()
    out_ap = out.ap()

    with tile.TileContext(nc) as tc:
        with (
            tc.tile_pool(name="const", bufs=1) as cpool,
            tc.tile_pool(name="xtg", bufs=2) as xtpool,
            tc.tile_pool(name="stgp", bufs=2) as stgpool,
            tc.tile_pool(name="selfp", bufs=2) as spool,
            tc.tile_pool(name="gb", bufs=GBUFS) as gpool,
            tc.tile_pool(name="addp", bufs=3) as addpool,
            tc.tile_pool(name="outp", bufs=2) as opool,
            tc.tile_pool(name="pacc", bufs=4, space="PSUM") as paccpool,
            tc.tile_pool(name="stgd", bufs=1, space="DRAM") as dstgpool,
        ):
            stg_t = [dstgpool.tile([TOTPOS, D], _FEAT_DT, name=f"stg{s}")
                     for s in range(NNBR)]
            w_sb = cpool.tile([P, 4 * KCHUNKS, D], _FEAT_DT)
            nc.sync.dma_start(
                out=w_sb[:], in_=w.ap().rearrange("(c p) n -> p c n", p=P))
            bias_sb = cpool.tile([1, D], _FEAT_DT)
            nc.sync.dma_start(out=bias_sb[:], in_=bias.ap())
            idx16_sb = cpool.tile([P, NNBR * NI16], i16)
            nc.sync.dma_start(out=idx16_sb[:], in_=idx16.ap())
            pos16_sb = cpool.tile([P, NNBR * NP16], i16)
            nc.sync.dma_start(out=pos16_sb[:], in_=pos16.ap())
            ones_sb = cpool.tile([1, P], _FEAT_DT)
            nc.gpsimd.memset(ones_sb[:], 1.0)

            for rr in range(repeat):
                # ---- Phase A ----
                for s in range(NNBR):
                    for r in range(NREG):
                        cap = CAPS[r]
                        wr = min(REGW, NPAD - r * REGW)
                        xt = xtpool.tile([P, KCHUNKS, cap], _FEAT_DT)
                        nc.gpsimd.dma_gather(
                            out_ap=xt[:],
                            in_ap=feats_ap[r * REGW:r * REGW + wr, :],
                            idxs_ap=idx16_sb[
                                :, s * NI16 + int(RBASE[r]) // 16:
                                s * NI16 + int(RBASE[r]) // 16 + cap // 16],
                            num_idxs=cap, num_idxs_reg=cap,
                            elem_size=D, transpose=True)
                        stg_sb = stgpool.tile([P, cap // P, D], _FEAT_DT)
                        for j in range(cap // P):
                            pacc = paccpool.tile([P, D], f32)
                            for ki in range(KCHUNKS):
                                nc.tensor.matmul(
                                    pacc[:],
                                    lhsT=xt[:, ki, j * P:(j + 1) * P],
                                    rhs=w_sb[:, (s + 1) * KCHUNKS + ki, :],
                                    start=(ki == 0), stop=(ki == 1))
                            if j % 2 == 0:
                                nc.vector.tensor_copy(out=stg_sb[:, j, :],
                                                      in_=pacc[:])
                            else:
                                nc.scalar.activation(
                                    stg_sb[:, j, :], pacc[:],
                                    mybir.ActivationFunctionType.Copy)
                        nc.sync.dma_start(
                            out=stg_t[s][int(RBASE[r]):int(RBASE[r]) + cap,
                                         :].rearrange("(k p) n -> p k n", p=P),
                            in_=stg_sb[:])

                # ---- Phase B ----
                for b in range(DBG_NBLOCKS or NBLOCKS):
                    r0 = b * G * P
                    gts = []
                    for s in range(NNBR):
                        gt = gpool.tile([P, G, D], _FEAT_DT, tag="gt")
                        nc.gpsimd.dma_gather(
                            out_ap=gt[:],
                            in_ap=stg_t[s][:, :],
                            idxs_ap=pos16_sb[
                                :, s * NP16 + b * (G * P) // 16:
                                s * NP16 + (b + 1) * (G * P) // 16],
                            num_idxs=G * P, num_idxs_reg=G * P,
                            elem_size=D, transpose=False)
                        gts.append(gt)
                    xself = []
                    for ki in range(KCHUNKS):
                        xs = spool.tile([P, G * P], _FEAT_DT, tag="xself")
                        nc.sync.dma_start(
                            out=xs[:],
                            in_=feats_ap[r0:r0 + G * P, ki * P:(ki + 1) * P],
                            transpose=True)
                        xself.append(xs)
                    t1 = addpool.tile([P, G, D], _FEAT_DT, tag="t1")
                    nc.vector.scalar_tensor_tensor(
                        out=t1[:], in0=gts[0][:], scalar=0.0, in1=gts[1][:],
                        op0=mybir.AluOpType.bypass, op1=mybir.AluOpType.add)
                    t2 = addpool.tile([P, G, D], _FEAT_DT, tag="t2")
                    nc.vector.scalar_tensor_tensor(
                        out=t2[:], in0=t1[:], scalar=0.0, in1=gts[2][:],
                        op0=mybir.AluOpType.bypass, op1=mybir.AluOpType.add)
                    outsb = opool.tile([P, G, D], mybir.dt.float32)
                    for g in range(G):
                        pacc = paccpool.tile([P, D], f32)
                        for ki in range(KCHUNKS):
                            nc.tensor.matmul(
                                pacc[:], lhsT=xself[ki][:, g * P:(g + 1) * P],
                                rhs=w_sb[:, ki, :],
                                start=(ki == 0), stop=False)
                        nc.tensor.matmul(
                            pacc[:], lhsT=ones_sb[:1, :], rhs=bias_sb[:1, :],
                            start=False, stop=True)
                        zt = addpool.tile([P, D], mybir.dt.float32, tag="zt")
                        nc.vector.scalar_tensor_tensor(
                            out=zt[:], in0=pacc[:], scalar=0.0,
                            in1=t2[:, g, :],
                            op0=mybir.AluOpType.bypass,
                            op1=mybir.AluOpType.add)
                        nc.scalar.activation(
                            outsb[:, g, :], zt[:],
                            mybir.ActivationFunctionType.Relu)
                    nc.sync.dma_start(
                        out=out_ap[r0:r0 + G * P, :].rearrange(
                            "(g p) n -> p g n", p=P),
                        in_=outsb[:],
                    )
    nc.compile()
    return nc


def _prepare_g2_extra(in_maps, next_indices, prev_indices, mate_indices):
    """Add idx16/pos16 tensors (region-bucketed int16 gather indices and
    un-sort positions) to each core's input map."""
    nbr = [np.asarray(next_indices), np.asarray(prev_indices),
           np.asarray(mate_indices)]
    NI16 = TOTPOS // 16
    NP16 = PAD_ROWS // 16
    for c in range(NCORES):
        base = c * ROWS_PER_CORE
        idx16 = np.zeros((16, NNBR * NI16), dtype=np.int16)
        pos16 = np.zeros((16, NNBR * NP16), dtype=np.int16)
        for s, I in enumerate(nbr):
            J = np.zeros(PAD_ROWS, dtype=np.int64)
            J[:ROWS_PER_CORE] = (I[base:base + ROWS_PER_CORE] - base) % NPAD
            reg = J // REGW
            rel = (J - reg * REGW).astype(np.int64)
            pos = np.zeros(PAD_ROWS, dtype=np.int64)
            relcat = np.zeros(TOTPOS, dtype=np.int16)
            for r in range(NREG):
                rows = np.flatnonzero(reg == r)
                n = len(rows)
                assert n <= CAPS[r], (c, s, r, n, CAPS[r])
                pos[rows] = RBASE[r] + np.arange(n)
                relcat[RBASE[r]:RBASE[r] + n] = rel[rows].astype(np.int16)
            idx16[:, s * NI16:(s + 1) * NI16] = relcat.reshape(
                NI16, 16).T
            pos16[:, s * NP16:(s + 1) * NP16] = pos.astype(np.int16).reshape(
                NP16, 16).T
        in_maps[c]["idx16"] = np.ascontiguousarray(np.tile(idx16, (8, 1)))
        in_maps[c]["pos16"] = np.ascontiguousarray(np.tile(pos16, (8, 1)))
        del in_maps[c]["idx"]
    return in_maps


def _prepare_in_maps(features, next_indices, prev_indices, mate_indices,
                     W_self, b_self, W_next, b_next, W_prev, b_prev,
                     W_mate, b_mate):
    feats = np.zeros((NPAD, D), dtype=_FEAT_NP)
    feats[:N] = np.asarray(features, dtype=np.float32).astype(_FEAT_NP)
    # Each core gets the table rotated so its own rows start at 0: the self
    # stream then reads a static slice [r0, r0+G*P) in an SPMD-identical
    # program.  Neighbor indices are remapped by (I - base) mod NPAD.

    w_cat = np.concatenate(
        [np.asarray(W_self, np.float32), np.asarray(W_next, np.float32),
         np.asarray(W_prev, np.float32), np.asarray(W_mate, np.float32)],
        axis=0).astype(_FEAT_NP)
    w_cat = np.ascontiguousarray(w_cat)
    b_tot = (np.asarray(b_self, np.float32) + np.asarray(b_next, np.float32)
             + np.asarray(b_prev, np.float32) + np.asarray(b_mate, np.float32))
    b_tot = np.ascontiguousarray(b_tot.reshape(1, D).astype(_FEAT_NP))

    nbr = [np.asarray(next_indices), np.asarray(prev_indices),
           np.asarray(mate_indices)]

    in_maps = []
    for c in range(NCORES):
        base = c * ROWS_PER_CORE
        feats_c = np.roll(feats, -base, axis=0) if base else feats
        # idx layout: [P, NBLOCKS, NNBR, G].
        # Local row r = b*G*P + g*P + p -> partition p of subtile (b, g).
        # Value: rotated neighbor index of local row r (0 for pad rows).
        idx_arr = np.zeros((P, NBLOCKS, NNBR, G), dtype=np.int32)
        for s, I in enumerate(nbr):
            loc = np.zeros(PAD_ROWS, dtype=np.int64)
            loc[:ROWS_PER_CORE] = (I[base:base + ROWS_PER_CORE] - base) % NPAD
            idx_arr[:, :, s, :] = (
                loc.reshape(NBLOCKS, G, P).transpose(2, 0, 1).astype(np.int32))
        idx_flat = np.ascontiguousarray(
            idx_arr.reshape(P, NBLOCKS * NNBR * G))
        in_maps.append({
            "features": feats_c,
            "w": w_cat,
            "bias": b_tot,
            "idx": idx_flat,
        })
    if DESIGN == "g2":
        _prepare_g2_extra(in_maps, next_indices, prev_indices, mate_indices)
    return in_maps


def _unpad_output(results):
    """Concatenate per-core padded outputs back to the full [N, D] array."""
    out = np.concatenate(
        [results[c]["out"][:ROWS_PER_CORE] for c in range(NCORES)], axis=0)
    return np.ascontiguousarray(out.astype(np.float32))


def kernel(**inputs) -> np.ndarray:
    in_maps = _prepare_in_maps(**inputs)
    nc = _build_nc()
    res = bass_utils.run_bass_kernel_spmd(
        nc, in_maps, core_ids=list(range(NCORES)))
    return _unpad_output(res.results)
